# revision 19
# baseline (speedup 1.0000x reference)
"""Trainium2 Bass kernel for 7x7 sliding-window self-similarity attention.

out[b,c,h,w] = sum_j softmax_j(x[h,w] * x[h+dh,w+dw]) * x[h+dh,w+dw]
over the 7x7 neighborhood (zero padding, pad=3).

Sharding: B*C = 256 independent 128x128 images, 32 images per core on 8
NeuronCores (pure data parallel, no collectives).

Per-core schedule: 2 sequential batches of 16 images; partition
p = rowblock(0..7)*16 + image(0..15), each holding a 28-row x 140-col
zero-padded fp32 slab (3920 contiguous floats). Every 7x7 shift is a flat
offset view; elementwise ops run on contiguous 1D runs spanning pad
columns (finite garbage there, never read). 2048 output pixels per
partition means PSUM (4096 fp32) can hold BOTH accumulators at once.

Score symmetry: e_{-d}[i] == e_d[i-d]; only 25 canonical score tiles are
computed on an extended halo run; mirrored contributions are views.

Numerator trick: sum_d e_d[i]*x[i+d] = (sum of t_d = e_d*s_d views)/x[i]
(s_d is the score itself); the final division by x cancels exactly:
out = acc_t / (x * sum_e).

Engines: DVE does score/t products + a few acc views + finale; ACT does
exp (twice when SUM_BF16: fp32 for t, bf16 for PE); TensorE accumulates
sum_e (bf16 moving) and most of acc_t (fp32 LOW_HIGH) into PSUM via
identity matmuls on its own SBUF ports; GpSimd stays idle (shares DVE's
second SBUF read port - concurrency is net-negative, measured 3x).
"""

import numpy as np

import concourse.bacc as bacc
import concourse.bass as bass  # noqa: F401
import concourse.tile as tile
from concourse import mybir
from concourse.bass_utils import run_bass_kernel_spmd

N_CORES = 8
F32 = mybir.dt.float32
BF16 = mybir.dt.bfloat16
MULT = mybir.AluOpType.mult
ADD = mybir.AluOpType.add

B, C, H, W = 4, 64, 128, 128
N_IMG_TOTAL = B * C
IMG_PER_CORE = N_IMG_TOTAL // N_CORES  # 32
BATCHES = 2
RB_N = 8                      # rowblocks per image within a batch
PAD = 6
MM_CHUNK = 512                # one PSUM bank of fp32

SUM_BF16 = True               # denominator matmuls in bf16 (else fp32 2-pass)
N_DVE_ACC_VIEWS = 5           # acc_t views handled by DVE per batch


def canonical_offsets():
    canon = [(0, 0)]
    canon += [(0, dj) for dj in range(1, 4)]
    canon += [(di, dj) for di in range(1, 4) for dj in range(-3, 4)]
    return canon


def view2d(ap, off, rows, cols, stride):
    """Strided [rows, cols] view at element offset `off` of a flat [P, L] AP."""
    a = ap.copy()
    pair_t = type(a.ap)
    part = list(a.ap)[0]
    a.ap = pair_t([list(part), [stride, rows], [1, cols]])
    a.offset = a.offset + off
    return a


def build_nc(n_img=IMG_PER_CORE, h=H, w=W):
    nb = n_img // BATCHES        # images per batch (16)
    br = h // RB_N               # rows per block (16)
    wp = w + 2 * PAD             # 140
    slab = br + 2 * PAD          # 28
    P = nb * RB_N                # 128

    nx = slab * wp               # 3920
    le = (br + 6) * wp + 8       # 3088 extended run
    soff = 3 * wp - 4
    la = br * wp                 # 2240 full-width run
    lc = br * w                  # 2048 compact output
    t0_off = 3 * wp + 4
    xq_off = 6 * wp
    mm_chunk = min(MM_CHUNK, lc)
    n_chunks = lc // mm_chunk    # 4
    rpc = mm_chunk // w          # rows per chunk (4)

    nc = bacc.Bacc("TRN2", target_bir_lowering=False, debug=False)
    x_in = nc.dram_tensor("x", [BATCHES, P, nx], F32, kind="ExternalInput")
    id_in = nc.dram_tensor("ident", [P, P], F32, kind="ExternalInput")
    y_out = nc.dram_tensor("y", [BATCHES, P, lc], F32, kind="ExternalOutput")

    canon = canonical_offsets()
    n_views = 2 * len(canon) - 1  # 49
    n_dve = min(N_DVE_ACC_VIEWS, n_views)
    n_pe_acc = n_views - n_dve

    with tile.TileContext(nc) as tc:
        with (
            tc.tile_pool(name="big", bufs=1) as big,
            tc.tile_pool(name="xp", bufs=2) as xpool,
            tc.tile_pool(name="sp", bufs=2) as spool,
            tc.tile_pool(name="ep", bufs=2) as epool,
            tc.tile_pool(name="tp", bufs=2) as tpool,
            tc.tile_pool(name="ac", bufs=2) as apool,
            tc.tile_pool(name="fin", bufs=2) as fin,
            tc.tile_pool(name="ps", bufs=1, space="PSUM") as ps,
        ):
            ident = big.tile([P, P], F32, tag="id")
            identb = big.tile([P, P], BF16, tag="idb")
            nc.sync.dma_start(out=ident[:], in_=id_in[:])
            nc.vector.tensor_copy(identb[:], ident[:])

            for b in range(BATCHES):
                x = xpool.tile([P, nx + 8], F32, tag="x")
                nc.sync.dma_start(out=x[:, :nx], in_=x_in[b])
                nc.vector.memset(x[:, nx:], 0.0)

                acc_s = apool.tile([P, la], F32, tag="acc")
                psum_s = ps.tile([P, lc], F32, tag="psS")
                psum_a = ps.tile([P, lc], F32, tag="psA")

                vs = 0   # sum-view counter
                va = 0   # PE acc-view counter
                vd = 0   # DVE acc-view counter
                for k, (di, dj) in enumerate(canon):
                    df = di * wp + dj
                    s = spool.tile([P, le], F32, tag="s")
                    e = epool.tile([P, le], F32, tag="e")

                    nc.vector.tensor_tensor(
                        out=s[:],
                        in0=x[:, soff:soff + le],
                        in1=x[:, soff + df:soff + df + le],
                        op=MULT,
                    )
                    nc.scalar.activation(
                        out=e[:], in_=s[:],
                        func=mybir.ActivationFunctionType.Exp,
                    )
                    if SUM_BF16:
                        eb = epool.tile([P, le], BF16, tag="eb")
                        nc.scalar.activation(
                            out=eb[:], in_=s[:],
                            func=mybir.ActivationFunctionType.Exp,
                        )
                        sum_src, sum_w = eb, identb
                    else:
                        sum_src, sum_w = e, ident

                    t = tpool.tile([P, le], F32, tag="t")
                    nc.vector.tensor_tensor(out=t[:], in0=e[:], in1=s[:],
                                            op=MULT)

                    offs = [t0_off]
                    if (di, dj) != (0, 0):
                        offs.append(t0_off - df)

                    for to in offs:
                        # denominator on PE
                        eo = to + PAD
                        for ci in range(n_chunks):
                            mv = view2d(sum_src[:], eo + ci * rpc * wp,
                                        rpc, w, wp)
                            nc.tensor.matmul(
                                psum_s[:, ci * mm_chunk:(ci + 1) * mm_chunk],
                                sum_w[:], mv,
                                start=(vs == 0), stop=(vs == n_views - 1),
                            )
                        vs += 1

                        # numerator: first n_dve views on DVE, rest on PE
                        if vd < n_dve:
                            tv = t[:, to:to + la]
                            if vd == 0:
                                nc.vector.tensor_copy(acc_s[:], tv)
                            else:
                                nc.vector.tensor_tensor(
                                    out=acc_s[:], in0=acc_s[:], in1=tv, op=ADD
                                )
                            vd += 1
                        else:
                            for ci in range(n_chunks):
                                mv = view2d(t[:], to + PAD + ci * rpc * wp,
                                            rpc, w, wp)
                                nc.tensor.matmul(
                                    psum_a[:, ci * mm_chunk:
                                           (ci + 1) * mm_chunk],
                                    ident[:], mv,
                                    start=(va == 0), stop=(va == n_pe_acc - 1),
                                )
                            va += 1

                # finale: out = (acc_s + psum_a) / (x * sum_e)
                den = fin.tile([P, lc], F32, tag="den")
                r = fin.tile([P, lc], F32, tag="r")
                scr = fin.tile([P, lc], F32, tag="scr")
                num = fin.tile([P, lc], F32, tag="num")
                xc = view2d(x[:], xq_off + PAD, br, w, wp)
                nc.vector.tensor_tensor(out=den[:], in0=psum_s[:], in1=xc,
                                        op=MULT)
                nc.vector.reciprocal_approx_accurate(
                    out=r[:], in_=den[:], scratch=scr[:]
                )
                av = view2d(acc_s[:], PAD, br, w, wp)
                nc.vector.tensor_tensor(out=num[:], in0=av, in1=psum_a[:],
                                        op=ADD)
                out_c = fin.tile([P, lc], F32, tag="den")
                nc.vector.tensor_tensor(out=out_c[:], in0=num[:], in1=r[:],
                                        op=MULT)
                nc.sync.dma_start(out=y_out[b], in_=out_c[:])
    nc.compile()
    return nc


_NC_CACHE = {}


def _get_nc():
    if "nc" not in _NC_CACHE:
        _NC_CACHE["nc"] = build_nc()
    return _NC_CACHE["nc"]


def make_slabs(imgs, h=H, w=W):
    """[n,h,w] fp32 -> [BATCHES, nb*RB_N, slab*wp] slab layout."""
    n = imgs.shape[0]
    nb = n // BATCHES
    br = h // RB_N
    slab = br + 2 * PAD
    xp = np.pad(imgs, ((0, 0), (PAD, PAD), (PAD, PAD)))
    rows = (np.arange(RB_N) * br)[:, None] + np.arange(slab)  # [8, slab]
    sl = xp[:, rows, :]                   # [n, 8, slab, wp]
    sl = sl.reshape(BATCHES, nb, RB_N, slab, -1).transpose(0, 2, 1, 3, 4)
    return np.ascontiguousarray(sl.reshape(BATCHES, RB_N * nb, -1))


def unslab_out(y, n_img, h=H, w=W):
    """[BATCHES, nb*RB_N, br*w] -> [n_img, h, w]."""
    nb = n_img // BATCHES
    br = h // RB_N
    y = y.reshape(BATCHES, RB_N, nb, br, w).transpose(0, 2, 1, 3, 4)
    return np.ascontiguousarray(y.reshape(n_img, h, w))


def run(x, **spmd_kwargs):
    nc = _get_nc()
    imgs = np.ascontiguousarray(np.asarray(x).reshape(N_IMG_TOTAL, H, W))
    imgs = imgs.astype(np.float32, copy=False)
    ident = np.eye(128, dtype=np.float32)
    in_maps = [
        {"x": make_slabs(imgs[i * IMG_PER_CORE:(i + 1) * IMG_PER_CORE]),
         "ident": ident}
        for i in range(N_CORES)
    ]
    res = run_bass_kernel_spmd(nc, in_maps, core_ids=list(range(N_CORES)),
                               **spmd_kwargs)
    out = np.concatenate(
        [unslab_out(res.results[i]["y"], IMG_PER_CORE) for i in range(N_CORES)],
        axis=0,
    )
    return out.reshape(B, C, H, W).astype(np.float32, copy=False), res


def kernel(x):
    out, _ = run(x)
    return out


# revision 20
# speedup vs baseline: 1.1818x; 1.1818x over previous
"""Trainium2 Bass kernel for 7x7 sliding-window self-similarity attention.

out[b,c,h,w] = sum_j softmax_j(x[h,w] * x[h+dh,w+dw]) * x[h+dh,w+dw]
over the 7x7 neighborhood (zero padding, pad=3).

Sharding: B*C = 256 independent 128x128 images, 32 images per core on 8
NeuronCores (pure data parallel, no collectives).

Per-core schedule: 2 sequential batches of 16 images; partition
p = rowblock(0..7)*16 + image(0..15), each holding a 28-row x 140-col
zero-padded fp32 slab (3920 contiguous floats). Every 7x7 shift is a flat
offset view; elementwise ops run on contiguous 1D runs spanning pad
columns (finite garbage there, never read). 2048 output pixels per
partition means PSUM (4096 fp32) can hold BOTH accumulators at once.

Score symmetry: e_{-d}[i] == e_d[i-d]; only 25 canonical score tiles are
computed on an extended halo run; mirrored contributions are views.

Numerator trick: sum_d e_d[i]*x[i+d] = (sum of t_d = e_d*s_d views)/x[i]
(s_d is the score itself); the final division by x cancels exactly:
out = acc_t / (x * sum_e).

Engines: DVE does score/t products + a few acc views + finale; ACT does
exp (twice when SUM_BF16: fp32 for t, bf16 for PE); TensorE accumulates
sum_e (bf16 moving) and most of acc_t (fp32 LOW_HIGH) into PSUM via
identity matmuls on its own SBUF ports; GpSimd stays idle (shares DVE's
second SBUF read port - concurrency is net-negative, measured 3x).
"""

import numpy as np

import concourse.bacc as bacc
import concourse.bass as bass  # noqa: F401
import concourse.tile as tile
from concourse import mybir
from concourse.bass_utils import run_bass_kernel_spmd

N_CORES = 8
F32 = mybir.dt.float32
BF16 = mybir.dt.bfloat16
MULT = mybir.AluOpType.mult
ADD = mybir.AluOpType.add

B, C, H, W = 4, 64, 128, 128
N_IMG_TOTAL = B * C
IMG_PER_CORE = N_IMG_TOTAL // N_CORES  # 32
BATCHES = 2
RB_N = 8                      # rowblocks per image within a batch
PAD = 6
MM_CHUNK = 512                # one PSUM bank of fp32

SUM_BF16 = True               # denominator matmuls in bf16 (else fp32 2-pass)
N_DVE_ACC_VIEWS = 7           # acc_t views handled by DVE per batch


def canonical_offsets():
    canon = [(0, 0)]
    canon += [(0, dj) for dj in range(1, 4)]
    canon += [(di, dj) for di in range(1, 4) for dj in range(-3, 4)]
    return canon


def view2d(ap, off, rows, cols, stride):
    """Strided [rows, cols] view at element offset `off` of a flat [P, L] AP."""
    a = ap.copy()
    pair_t = type(a.ap)
    part = list(a.ap)[0]
    a.ap = pair_t([list(part), [stride, rows], [1, cols]])
    a.offset = a.offset + off
    return a


def build_nc(n_img=IMG_PER_CORE, h=H, w=W):
    nb = n_img // BATCHES        # images per batch (16)
    br = h // RB_N               # rows per block (16)
    wp = w + 2 * PAD             # 140
    slab = br + 2 * PAD          # 28
    P = nb * RB_N                # 128

    nx = slab * wp               # 3920
    le = (br + 6) * wp + 8       # 3088 extended run
    soff = 3 * wp - 4
    la = br * wp                 # 2240 full-width run
    lc = br * w                  # 2048 compact output
    t0_off = 3 * wp + 4
    xq_off = 6 * wp
    mm_chunk = min(MM_CHUNK, lc)
    n_chunks = lc // mm_chunk    # 4
    rpc = mm_chunk // w          # rows per chunk (4)

    nc = bacc.Bacc("TRN2", target_bir_lowering=False, debug=False)
    x_in = nc.dram_tensor("x", [BATCHES, P, nx], F32, kind="ExternalInput")
    id_in = nc.dram_tensor("ident", [P, P], F32, kind="ExternalInput")
    y_out = nc.dram_tensor("y", [BATCHES, P, lc], F32, kind="ExternalOutput")

    canon = canonical_offsets()
    n_views = 2 * len(canon) - 1  # 49
    n_dve = min(N_DVE_ACC_VIEWS, n_views)
    n_pe_acc = n_views - n_dve

    with tile.TileContext(nc) as tc:
        with (
            tc.tile_pool(name="big", bufs=1) as big,
            tc.tile_pool(name="xp", bufs=2) as xpool,
            tc.tile_pool(name="sp", bufs=2) as spool,
            tc.tile_pool(name="ep", bufs=2) as epool,
            tc.tile_pool(name="tp", bufs=2) as tpool,
            tc.tile_pool(name="ac", bufs=2) as apool,
            tc.tile_pool(name="fin", bufs=2) as fin,
            tc.tile_pool(name="ps", bufs=1, space="PSUM") as ps,
        ):
            ident = big.tile([P, P], F32, tag="id")
            identb = big.tile([P, P], BF16, tag="idb")
            nc.sync.dma_start(out=ident[:], in_=id_in[:])
            nc.vector.tensor_copy(identb[:], ident[:])

            for b in range(BATCHES):
                x = xpool.tile([P, nx + 8], F32, tag="x")
                nc.sync.dma_start(out=x[:, :nx], in_=x_in[b])
                nc.vector.memset(x[:, nx:], 0.0)

                acc_s = apool.tile([P, la], F32, tag="acc")
                psum_s = ps.tile([P, lc], F32, tag="psS")
                psum_a = ps.tile([P, lc], F32, tag="psA")

                vs = 0   # sum-view counter
                va = 0   # PE acc-view counter
                vd = 0   # DVE acc-view counter
                for k, (di, dj) in enumerate(canon):
                    df = di * wp + dj
                    s = spool.tile([P, le], F32, tag="s")
                    e = epool.tile([P, le], F32, tag="e")

                    nc.vector.tensor_tensor(
                        out=s[:],
                        in0=x[:, soff:soff + le],
                        in1=x[:, soff + df:soff + df + le],
                        op=MULT,
                    )
                    nc.scalar.activation(
                        out=e[:], in_=s[:],
                        func=mybir.ActivationFunctionType.Exp,
                    )
                    if SUM_BF16:
                        eb = epool.tile([P, le], BF16, tag="eb")
                        nc.scalar.activation(
                            out=eb[:], in_=s[:],
                            func=mybir.ActivationFunctionType.Exp,
                        )
                        sum_src, sum_w = eb, identb
                    else:
                        sum_src, sum_w = e, ident

                    t = tpool.tile([P, le], F32, tag="t")
                    nc.vector.tensor_tensor(out=t[:], in0=e[:], in1=s[:],
                                            op=MULT)

                    offs = [t0_off]
                    if (di, dj) != (0, 0):
                        offs.append(t0_off - df)

                    for to in offs:
                        # denominator on PE
                        eo = to + PAD
                        for ci in range(n_chunks):
                            mv = view2d(sum_src[:], eo + ci * rpc * wp,
                                        rpc, w, wp)
                            nc.tensor.matmul(
                                psum_s[:, ci * mm_chunk:(ci + 1) * mm_chunk],
                                sum_w[:], mv,
                                start=(vs == 0), stop=(vs == n_views - 1),
                            )
                        vs += 1

                        # numerator: first n_dve views on DVE, rest on PE
                        if vd < n_dve:
                            tv = t[:, to:to + la]
                            if vd == 0:
                                nc.vector.tensor_copy(acc_s[:], tv)
                            else:
                                nc.vector.tensor_tensor(
                                    out=acc_s[:], in0=acc_s[:], in1=tv, op=ADD
                                )
                            vd += 1
                        else:
                            for ci in range(n_chunks):
                                mv = view2d(t[:], to + PAD + ci * rpc * wp,
                                            rpc, w, wp)
                                nc.tensor.matmul(
                                    psum_a[:, ci * mm_chunk:
                                           (ci + 1) * mm_chunk],
                                    ident[:], mv,
                                    start=(va == 0), stop=(va == n_pe_acc - 1),
                                )
                            va += 1

                # finale: out = (acc_s + psum_a) / (x * sum_e)
                den = fin.tile([P, lc], F32, tag="den")
                r = fin.tile([P, lc], F32, tag="r")
                scr = fin.tile([P, lc], F32, tag="scr")
                num = fin.tile([P, lc], F32, tag="num")
                xc = view2d(x[:], xq_off + PAD, br, w, wp)
                nc.vector.tensor_tensor(out=den[:], in0=psum_s[:], in1=xc,
                                        op=MULT)
                nc.vector.reciprocal_approx_accurate(
                    out=r[:], in_=den[:], scratch=scr[:]
                )
                av = view2d(acc_s[:], PAD, br, w, wp)
                nc.vector.tensor_tensor(out=num[:], in0=av, in1=psum_a[:],
                                        op=ADD)
                out_c = fin.tile([P, lc], F32, tag="den")
                nc.vector.tensor_tensor(out=out_c[:], in0=num[:], in1=r[:],
                                        op=MULT)
                nc.sync.dma_start(out=y_out[b], in_=out_c[:])
    nc.compile()
    return nc


_NC_CACHE = {}


def _get_nc():
    if "nc" not in _NC_CACHE:
        _NC_CACHE["nc"] = build_nc()
    return _NC_CACHE["nc"]


def make_slabs(imgs, h=H, w=W):
    """[n,h,w] fp32 -> [BATCHES, nb*RB_N, slab*wp] slab layout."""
    n = imgs.shape[0]
    nb = n // BATCHES
    br = h // RB_N
    slab = br + 2 * PAD
    xp = np.pad(imgs, ((0, 0), (PAD, PAD), (PAD, PAD)))
    rows = (np.arange(RB_N) * br)[:, None] + np.arange(slab)  # [8, slab]
    sl = xp[:, rows, :]                   # [n, 8, slab, wp]
    sl = sl.reshape(BATCHES, nb, RB_N, slab, -1).transpose(0, 2, 1, 3, 4)
    return np.ascontiguousarray(sl.reshape(BATCHES, RB_N * nb, -1))


def unslab_out(y, n_img, h=H, w=W):
    """[BATCHES, nb*RB_N, br*w] -> [n_img, h, w]."""
    nb = n_img // BATCHES
    br = h // RB_N
    y = y.reshape(BATCHES, RB_N, nb, br, w).transpose(0, 2, 1, 3, 4)
    return np.ascontiguousarray(y.reshape(n_img, h, w))


def run(x, **spmd_kwargs):
    nc = _get_nc()
    imgs = np.ascontiguousarray(np.asarray(x).reshape(N_IMG_TOTAL, H, W))
    imgs = imgs.astype(np.float32, copy=False)
    ident = np.eye(128, dtype=np.float32)
    in_maps = [
        {"x": make_slabs(imgs[i * IMG_PER_CORE:(i + 1) * IMG_PER_CORE]),
         "ident": ident}
        for i in range(N_CORES)
    ]
    res = run_bass_kernel_spmd(nc, in_maps, core_ids=list(range(N_CORES)),
                               **spmd_kwargs)
    out = np.concatenate(
        [unslab_out(res.results[i]["y"], IMG_PER_CORE) for i in range(N_CORES)],
        axis=0,
    )
    return out.reshape(B, C, H, W).astype(np.float32, copy=False), res


def kernel(x):
    out, _ = run(x)
    return out
